# revision 32
# baseline (speedup 1.0000x reference)
"""GAT (DGL GATConv) over complete per-doc graphs — Trainium2 Bass kernel.

Problem: nn_CompletedSentenceGraph (gnn_message_passing).
  64 docs x 512 sentences, HIDDEN=256, HEADS=4, D=256.
  h = (x @ W).reshape(B,S,H,D)
  el/er = einsum(h, attn_l/attn_r)
  e[b,s,t,h] = leaky_relu(el[s]+er[t], 0.2); alpha = softmax over s
  out = einsum(alpha, h) + bias; return mean over heads  -> [N, 256]

Sharding: data-parallel over docs, 8 docs per core on 8 cores.

Math tricks used on-device:
  * exp(lrelu(x)) = max(exp(x), exp(0.2 x)); with x = el_s + er_t both exps
    are rank-1, so per (src,dst) scores need only ONE vector op:
       expe'[s,t] = max(a_s, c_s * m_t),  a=exp(el), c=exp(0.2 el),
       m=exp(-0.8 er)   (score scaled by 1/exp(er_t) per-dst; softmax is
    invariant to per-dst scaling).
  * el/er are computed inside fused matmuls via an augmented weight matrix
    WLR = W @ ALR (ALR block-diagonal from attn_l/attn_r), built on device.
  * Z (softmax denominator) comes free as a ones-column appended to the
    aggregation matmul's rhs; ones are 4.0 so 1/Z also folds the 1/H
    head-mean factor.

Perf structure (v4):
  * PE executes its queue in order, so the doc loop is software-pipelined:
    agg(d-1) is emitted AFTER doc d's pt8/pc/proj matmuls.  While PE runs
    agg(d-1) (~7us), DVE/Pool generate expe(d) and Act does the h-copies
    of proj(d) — so agg(d) can start immediately after.
  * PSUM rings: pa-ring also hosts pt8, pb-ring also hosts pc (saves 2
    banks), leaving pa/pb double-buffered + two 2-bank agg tiles (head
    pairs 0/1 and 2/3) whose drains overlap each other's matmuls.
  * Setup weight transposes go through the DMA XBAR, and setup DMAs use
    the SWDGE (gpsimd) path so the per-doc x pipeline owns SP/HWDGE.
"""

from contextlib import ExitStack

import numpy as np

import concourse.mybir as mybir
import concourse.tile as tile
from concourse import bacc
from concourse.bass_utils import run_bass_kernel_spmd

F32 = mybir.dt.float32
BF16 = mybir.dt.bfloat16
AX = mybir.AluOpType

NUM_DOCS = 64
S = 512          # sentences per doc
K = 256          # hidden
H = 4            # heads
D = 256          # per-head out feats
N_CORES = 8
DPC = NUM_DOCS // N_CORES  # docs per core
P = 128

SS = S // P      # 4 s-subtiles per doc
KC = K // P      # 2 k-chunks
DC = S // P      # 4 dst chunks

EXP = mybir.ActivationFunctionType.Exp
CPY = mybir.ActivationFunctionType.Copy

# expe engine split: 12 DVE / 4 Pool (index = h*SS + ss, h-major order)
EXPE_ENG = ["v"] * 16
for _i in (1, 6, 9, 14):
    EXPE_ENG[_i] = "p"


def gat_tile_kernel(tc, x, w, al, ar, bias_d, out):
    nc = tc.nc

    stack = ExitStack()
    with stack:
        consts = stack.enter_context(tc.tile_pool(name="consts", bufs=1))
        # ---- load host-precomputed weight constants (3 small DMAs) ----
        # w:    [128, 2, 1024] bf16  (k on partitions)
        # al (reused slot): wlr [128, 2, 8] bf16 = W @ ALR, k on partitions
        # ar (reused slot): bias_b [128, 256] f32 broadcast of the head-mean
        w_bf = consts.tile([P, KC, H * D], BF16)
        nc.scalar.dma_start(out=w_bf, in_=w)
        wlr_bf = consts.tile([P, KC, 8], BF16)
        nc.scalar.dma_start(out=wlr_bf, in_=al)
        bias_b = consts.tile([P, D], F32)
        nc.scalar.dma_start(out=bias_b, in_=ar)

        # ---------------- per-doc software-pipelined loop ----------------
        with tc.tile_pool(name="xp", bufs=2) as xp, \
             tc.tile_pool(name="xtp", bufs=2) as xtp, \
             tc.tile_pool(name="hp", bufs=2) as hp, \
             tc.tile_pool(name="ep", bufs=2) as ep, \
             tc.tile_pool(name="mp", bufs=2) as mp, \
             tc.tile_pool(name="sp", bufs=2) as sp, \
             tc.tile_pool(name="op", bufs=2) as op_pool, \
             tc.tile_pool(name="drp", bufs=2, space="DRAM") as drp, \
             tc.tile_pool(name="ps_a", bufs=2, space="PSUM") as ps_a, \
             tc.tile_pool(name="ps_b", bufs=2, space="PSUM") as ps_b, \
             tc.tile_pool(name="ps_agg", bufs=1, space="PSUM") as ps_agg:

            prev = None  # tiles of doc d-1 whose agg is still pending

            def emit_front(d):
                """x load/transpose, logit chain, m_b — everything the
                expe generation of doc d needs, but NOT proj/agg."""
                xd = x[d * S:(d + 1) * S, :]
                x_sb = xp.tile([P, SS, K], F32)
                nc.sync.dma_start(out=x_sb,
                                  in_=xd.rearrange("(ss p) k -> p ss k", p=P))
                x_bf = xp.tile([P, SS, K], BF16, tag="xbf")
                nc.gpsimd.tensor_copy(out=x_bf, in_=x_sb)
                # transpose straight from SBUF via the DMA XBAR, one
                # [128s x 128k] tile per (ss, kc) — no DRAM round trip
                xt_bf = xtp.tile([P, KC, S], BF16)
                for ss in range(SS):
                    for kc in range(KC):
                        nc.sync.dma_start_transpose(
                            xt_bf[:, kc, ss * P:(ss + 1) * P],
                            x_bf[:, ss, kc * P:(kc + 1) * P])

                # pt8 (er/el rows) in the pa ring: [8, 512] slice of a pa slot
                pt8_t = ps_a.tile([P, S], F32, tag="pa")
                pt8 = pt8_t[0:8, :]
                for kc in range(KC):
                    nc.tensor.matmul(pt8, lhsT=wlr_bf[:, kc, :], rhs=xt_bf[:, kc, :],
                                     start=(kc == 0), stop=(kc == KC - 1))
                # pc (el per source) in the pb ring
                pc_t = ps_b.tile([P, S], F32, tag="pb")
                pc_all = pc_t[:, 0:SS * 8].rearrange("p (ss c) -> p ss c", c=8)
                for ss in range(SS):
                    for kc in range(KC):
                        nc.tensor.matmul(pc_all[:, ss, :],
                                         lhsT=xt_bf[:, kc, ss * P:(ss + 1) * P],
                                         rhs=wlr_bf[:, kc, :],
                                         start=(kc == 0), stop=(kc == KC - 1))

                # m_row = exp(-0.8*er); collapse; two 2-head broadcasts
                m_row = sp.tile([4, S], BF16, tag="mrow")
                nc.scalar.activation(out=m_row, in_=pt8[0:4, :], func=EXP,
                                     scale=-0.8)
                m_row4 = sp.tile([1, H, S], BF16, tag="mrow4")
                nc.scalar.dma_start(out=m_row4, in_=m_row[:, None, :])
                m_b = mp.tile([P, H, S], BF16)
                nc.gpsimd.partition_broadcast(m_b[:, 0:2, :], m_row4[:, 0:2, :])
                nc.gpsimd.partition_broadcast(m_b[:, 2:4, :], m_row4[:, 2:4, :])

                # elr + per-source exp scalars
                elr = sp.tile([P, SS, 8], F32, tag="elr")
                nc.vector.tensor_copy(out=elr, in_=pc_all)
                a_bf = sp.tile([P, SS, H], F32, tag="abf")
                c_bf = sp.tile([P, SS, H], F32, tag="cbf")
                nc.scalar.activation(out=a_bf, in_=elr[:, :, 4:8], func=EXP)
                nc.scalar.activation(out=c_bf, in_=elr[:, :, 4:8], func=EXP,
                                     scale=0.2)
                return dict(xt_bf=xt_bf, m_b=m_b, a_bf=a_bf, c_bf=c_bf)

            def emit_proj(ctx):
                """pa/pb matmuls + h-copies (Act) into ha_all."""
                xt_bf = ctx["xt_bf"]
                ha_all = hp.tile([P, SS, H, 257], BF16)
                nc.gpsimd.memset(ha_all[:, :, :, 256:257], 4.0)
                for ss in range(SS):
                    pa = ps_a.tile([P, S], F32, tag="pa")
                    pb = ps_b.tile([P, S], F32, tag="pb")
                    for kc in range(KC):
                        lt = xt_bf[:, kc, ss * P:(ss + 1) * P]
                        st, sp_ = (kc == 0), (kc == KC - 1)
                        nc.tensor.matmul(pa, lhsT=lt, rhs=w_bf[:, kc, 0:512],
                                         start=st, stop=sp_)
                        nc.tensor.matmul(pb, lhsT=lt, rhs=w_bf[:, kc, 512:1024],
                                         start=st, stop=sp_)
                    nc.scalar.copy(out=ha_all[:, ss, 0:2, 0:256],
                                   in_=pa.rearrange("p (h dd) -> p h dd", h=2))
                    nc.scalar.copy(out=ha_all[:, ss, 2:4, 0:256],
                                   in_=pb.rearrange("p (h dd) -> p h dd", h=2))
                ctx["ha_all"] = ha_all

            def emit_expe_chunk(ctx, chunk):
                """expe tiles for (h, ss) pairs in `chunk` (h-major index)."""
                expe = ctx["expe"]
                for idx in chunk:
                    h, ss = idx // SS, idx % SS
                    eng = nc.vector if EXPE_ENG[idx] == "v" else nc.gpsimd
                    eng.tensor_scalar(
                        out=expe[:, h, ss, :],
                        in0=ctx["m_b"][:, h, :],
                        scalar1=ctx["c_bf"][:, ss, h:h + 1],
                        scalar2=ctx["a_bf"][:, ss, h:h + 1],
                        op0=AX.mult, op1=AX.max)

            def emit_agg_dc(ctx, dc):
                """aggregation matmuls + normalize for one dst chunk of the
                PREVIOUS doc."""
                expe, ha_all = ctx["expe"], ctx["ha_all"]
                puA = ps_agg.tile([P, 2, S], F32, tag="puA")
                puB = ps_agg.tile([P, 2, S], F32, tag="puB")
                for hh, pu in ((0, puA), (2, puB)):
                    for h in (hh, hh + 1):
                        for sc in range(SS):
                            nc.tensor.matmul(pu[:, h - hh, 0:257],
                                             lhsT=expe[:, h, sc, dc * P:(dc + 1) * P],
                                             rhs=ha_all[:, sc, h, :],
                                             start=(sc == 0), stop=(sc == SS - 1))
                rzA = sp.tile([P, 2], F32, tag="rzA")
                nc.vector.reciprocal(out=rzA, in_=puA[:, :, 256])
                tmp = sp.tile([P, 2, D], BF16, tag="ntmp")
                nc.scalar.activation(out=tmp[:, 0, :], in_=puA[:, 0, 0:256],
                                     func=CPY, scale=rzA[:, 0:1])
                nc.vector.tensor_scalar(out=tmp[:, 1, :], in0=puA[:, 1, 0:256],
                                        scalar1=rzA[:, 1:2], scalar2=None,
                                        op0=AX.mult)
                rzB = sp.tile([P, 2], F32, tag="rzB")
                nc.vector.reciprocal(out=rzB, in_=puB[:, :, 256])
                tmp2 = sp.tile([P, D], BF16, tag="tmp2")
                nc.scalar.activation(out=tmp2, in_=puB[:, 0, 0:256],
                                     func=CPY, scale=rzB[:, 0:1])
                accd = sp.tile([P, D], F32, tag="accd")
                nc.vector.scalar_tensor_tensor(
                    out=accd, in0=puB[:, 1, 0:256], scalar=rzB[:, 1:2],
                    in1=bias_b, op0=AX.mult, op1=AX.add)
                t01 = sp.tile([P, D], BF16, tag="t01")
                nc.vector.tensor_add(out=t01, in0=tmp[:, 0, :], in1=tmp[:, 1, :])
                t012 = sp.tile([P, D], BF16, tag="t012")
                nc.vector.tensor_add(out=t012, in0=t01, in1=tmp2)
                nc.vector.tensor_add(out=ctx["out_sb"][:, dc, :], in0=t012, in1=accd)

            # expe emission chunks: first 8 tiles (heads 0,1) up front so
            # agg dc0 of THIS doc is unblocked early next iteration; the
            # rest interleave with the previous doc's agg post per dc.
            CHUNKS = [list(range(0, 6)), list(range(6, 10)),
                      list(range(10, 13)), list(range(13, 16)), []]

            for d in range(DPC):
                ctx = emit_front(d)
                expe_t = ep.tile([P, H, SS, S], BF16, tag="expe")
                ctx["expe"] = expe_t
                ctx["out_sb"] = None
                # proj(d) first: its Act h-copies then run during agg(d-1)
                # with a full doc of slack; agg(d-1) on PE overlaps expe(d)
                # generation on DVE/Pool.
                emit_proj(ctx)
                if prev is None:
                    for ch in CHUNKS:
                        emit_expe_chunk(ctx, ch)
                else:
                    out_sb_t = op_pool.tile([P, DC, D], F32, tag="osb")
                    prev["out_sb"] = out_sb_t
                    emit_expe_chunk(ctx, CHUNKS[0])
                    for dc in range(DC):
                        emit_agg_dc(prev, dc)
                        emit_expe_chunk(ctx, CHUNKS[dc + 1])
                    nc.scalar.dma_start(
                        out=out[(d - 1) * S:d * S, :]
                            .rearrange("(dc p) dd -> p dc dd", p=P),
                        in_=prev["out_sb"])
                prev = ctx

            # drain the last doc
            out_sb_last = op_pool.tile([P, DC, D], F32, tag="osb")
            prev["out_sb"] = out_sb_last
            for dc in range(DC):
                emit_agg_dc(prev, dc)
            nc.scalar.dma_start(
                out=out[(DPC - 1) * S:DPC * S, :]
                    .rearrange("(dc p) dd -> p dc dd", p=P),
                in_=prev["out_sb"])


_NC_CACHE = None


def build_nc():
    global _NC_CACHE
    if _NC_CACHE is not None:
        return _NC_CACHE
    nc = bacc.Bacc("TRN2", target_bir_lowering=False, debug=False,
                   num_devices=N_CORES)
    x = nc.dram_tensor("x", [DPC * S, K], F32, kind="ExternalInput")
    w = nc.dram_tensor("w", [P, KC, H * D], BF16, kind="ExternalInput")
    al = nc.dram_tensor("al", [P, KC, 8], BF16, kind="ExternalInput")
    ar = nc.dram_tensor("ar", [P, D], F32, kind="ExternalInput")
    bias_d = nc.dram_tensor("bias", [H * D], F32, kind="ExternalInput")
    out = nc.dram_tensor("out", [DPC * S, K], F32, kind="ExternalOutput")
    with tile.TileContext(nc) as tc:
        gat_tile_kernel(tc, x.ap(), w.ap(), al.ap(), ar.ap(), bias_d.ap(), out.ap())
    nc.compile()
    _NC_CACHE = nc
    return nc


def _host_weight_prep(W, attn_l, attn_r, bias):
    """Device-layout weight constants (host-side weight preprocessing).

    Returns (w_bf [128,2,1024] bf16, wlr_bf [128,2,8] bf16,
    bias_b [128,256] f32).  wlr col layout: cols 0..3 = W @ attn_r per head
    (er), cols 4..7 = W @ attn_l (el) — k rows split [kc, p]."""
    import ml_dtypes
    Wd = W.astype(np.float64)
    w_bf = W.astype(ml_dtypes.bfloat16).reshape(KC, P, H * D) \
        .transpose(1, 0, 2).copy()
    Wr = Wd.reshape(K, H, D)
    wlr = np.empty((K, 8), dtype=np.float64)
    wlr[:, 0:4] = np.einsum("khd,hd->kh", Wr, attn_r.astype(np.float64))
    wlr[:, 4:8] = np.einsum("khd,hd->kh", Wr, attn_l.astype(np.float64))
    wlr_bf = wlr.astype(ml_dtypes.bfloat16).reshape(KC, P, 8) \
        .transpose(1, 0, 2).copy()
    bias_mean = 0.25 * bias.astype(np.float64).reshape(H, D).sum(axis=0)
    bias_b = np.broadcast_to(bias_mean.astype(np.float32), (P, D)).copy()
    return w_bf, wlr_bf, bias_b


def kernel(sent_feature, W, attn_l, attn_r, bias, num_docs=NUM_DOCS, **_unused):
    sent_feature = np.asarray(sent_feature, dtype=np.float32)
    W = np.asarray(W, dtype=np.float32)
    attn_l = np.asarray(attn_l, dtype=np.float32)
    attn_r = np.asarray(attn_r, dtype=np.float32)
    bias = np.asarray(bias, dtype=np.float32)
    w_bf, wlr_bf, bias_b = _host_weight_prep(W, attn_l, attn_r, bias)

    nc = build_nc()
    in_maps = []
    rows = DPC * S
    for c in range(N_CORES):
        in_maps.append({
            "x": sent_feature[c * rows:(c + 1) * rows],
            "w": w_bf, "al": wlr_bf, "ar": bias_b, "bias": bias,
        })
    res = run_bass_kernel_spmd(nc, in_maps, core_ids=list(range(N_CORES)))
    out = np.concatenate([res.results[c]["out"] for c in range(N_CORES)], axis=0)
    return out.astype(np.float32)


# revision 33
# speedup vs baseline: 1.2376x; 1.2376x over previous
"""GAT (DGL GATConv) over complete per-doc graphs — Trainium2 Bass kernel.

Problem: nn_CompletedSentenceGraph (gnn_message_passing).
  64 docs x 512 sentences, HIDDEN=256, HEADS=4, D=256.
  h = (x @ W).reshape(B,S,H,D)
  el/er = einsum(h, attn_l/attn_r)
  e[b,s,t,h] = leaky_relu(el[s]+er[t], 0.2); alpha = softmax over s
  out = einsum(alpha, h) + bias; return mean over heads  -> [N, 256]

Sharding: data-parallel over docs, 8 docs per core on 8 cores.

Math tricks used on-device:
  * exp(lrelu(x)) = max(exp(x), exp(0.2 x)); with x = el_s + er_t both exps
    are rank-1, so per (src,dst) scores need only ONE vector op:
       expe'[s,t] = max(a_s, c_s * m_t),  a=exp(el), c=exp(0.2 el),
       m=exp(-0.8 er)   (score scaled by 1/exp(er_t) per-dst; softmax is
    invariant to per-dst scaling).
  * el/er are computed inside fused matmuls via an augmented weight matrix
    WLR = W @ ALR (ALR block-diagonal from attn_l/attn_r), built on device.
  * Z (softmax denominator) comes free as a ones-column appended to the
    aggregation matmul's rhs; ones are 4.0 so 1/Z also folds the 1/H
    head-mean factor.

Perf structure (v4):
  * PE executes its queue in order, so the doc loop is software-pipelined:
    agg(d-1) is emitted AFTER doc d's pt8/pc/proj matmuls.  While PE runs
    agg(d-1) (~7us), DVE/Pool generate expe(d) and Act does the h-copies
    of proj(d) — so agg(d) can start immediately after.
  * PSUM rings: pa-ring also hosts pt8, pb-ring also hosts pc (saves 2
    banks), leaving pa/pb double-buffered + two 2-bank agg tiles (head
    pairs 0/1 and 2/3) whose drains overlap each other's matmuls.
  * Setup weight transposes go through the DMA XBAR, and setup DMAs use
    the SWDGE (gpsimd) path so the per-doc x pipeline owns SP/HWDGE.
"""

from contextlib import ExitStack

import numpy as np

import concourse.mybir as mybir
import concourse.tile as tile
from concourse import bacc
from concourse.bass_utils import run_bass_kernel_spmd

F32 = mybir.dt.float32
BF16 = mybir.dt.bfloat16
AX = mybir.AluOpType

NUM_DOCS = 64
S = 512          # sentences per doc
K = 256          # hidden
H = 4            # heads
D = 256          # per-head out feats
N_CORES = 8
DPC = NUM_DOCS // N_CORES  # docs per core
P = 128

SS = S // P      # 4 s-subtiles per doc
KC = K // P      # 2 k-chunks
DC = S // P      # 4 dst chunks

EXP = mybir.ActivationFunctionType.Exp
CPY = mybir.ActivationFunctionType.Copy

# expe engine split: 12 DVE / 4 Pool (index = h*SS + ss, h-major order)
EXPE_ENG = ["v"] * 16
for _i in (1, 6, 9, 14):
    EXPE_ENG[_i] = "p"


def gat_tile_kernel(tc, x, w, al, ar, bias_d, out):
    nc = tc.nc

    stack = ExitStack()
    with stack:
        consts = stack.enter_context(tc.tile_pool(name="consts", bufs=1))
        # ---- load host-precomputed weight constants (3 small DMAs) ----
        # w:    [128, 2, 1024] bf16  (k on partitions)
        # al (reused slot): wlr [128, 2, 8] bf16 = W @ ALR, k on partitions
        # ar (reused slot): bias_b [128, 256] f32 broadcast of the head-mean
        w_bf = consts.tile([P, KC, H * D], BF16)
        nc.scalar.dma_start(out=w_bf, in_=w)
        wlr_bf = consts.tile([P, KC, 8], BF16)
        nc.scalar.dma_start(out=wlr_bf, in_=al)
        bias_b = consts.tile([P, D], F32)
        nc.scalar.dma_start(out=bias_b, in_=ar)

        # ---------------- per-doc software-pipelined loop ----------------
        with tc.tile_pool(name="xp", bufs=2) as xp, \
             tc.tile_pool(name="xtp", bufs=2) as xtp, \
             tc.tile_pool(name="hp", bufs=2) as hp, \
             tc.tile_pool(name="ep", bufs=2) as ep, \
             tc.tile_pool(name="mp", bufs=2) as mp, \
             tc.tile_pool(name="sp", bufs=2) as sp, \
             tc.tile_pool(name="op", bufs=2) as op_pool, \
             tc.tile_pool(name="drp", bufs=2, space="DRAM") as drp, \
             tc.tile_pool(name="ps_a", bufs=2, space="PSUM") as ps_a, \
             tc.tile_pool(name="ps_b", bufs=2, space="PSUM") as ps_b, \
             tc.tile_pool(name="ps_agg", bufs=1, space="PSUM") as ps_agg:

            prev = None  # tiles of doc d-1 whose agg is still pending

            def emit_front(d):
                """x load/transpose, logit chain, m_b — everything the
                expe generation of doc d needs, but NOT proj/agg."""
                xd = x[d * S:(d + 1) * S, :]
                x_sb = xp.tile([P, SS, K], F32)
                nc.sync.dma_start(out=x_sb,
                                  in_=xd.rearrange("(ss p) k -> p ss k", p=P))
                x_bf = xp.tile([P, SS, K], BF16, tag="xbf")
                nc.gpsimd.tensor_copy(out=x_bf, in_=x_sb)
                xdr = drp.tile([S, K], BF16)
                nc.sync.dma_start(out=xdr.rearrange("(ss p) k -> p ss k", p=P),
                                  in_=x_bf)
                xt_bf = xtp.tile([P, KC, S], BF16)
                for kc in range(KC):
                    nc.sync.dma_start_transpose(xt_bf[:, kc, :],
                                                xdr[:, kc * P:(kc + 1) * P])

                # pt8 (er/el rows) in the pa ring: [8, 512] slice of a pa slot
                pt8_t = ps_a.tile([P, S], F32, tag="pa")
                pt8 = pt8_t[0:8, :]
                for kc in range(KC):
                    nc.tensor.matmul(pt8, lhsT=wlr_bf[:, kc, :], rhs=xt_bf[:, kc, :],
                                     start=(kc == 0), stop=(kc == KC - 1))
                # pc (el per source) in the pb ring
                pc_t = ps_b.tile([P, S], F32, tag="pb")
                pc_all = pc_t[:, 0:SS * 8].rearrange("p (ss c) -> p ss c", c=8)
                for ss in range(SS):
                    for kc in range(KC):
                        nc.tensor.matmul(pc_all[:, ss, :],
                                         lhsT=xt_bf[:, kc, ss * P:(ss + 1) * P],
                                         rhs=wlr_bf[:, kc, :],
                                         start=(kc == 0), stop=(kc == KC - 1))

                # m_row = exp(-0.8*er); collapse; two 2-head broadcasts
                m_row = sp.tile([4, S], BF16, tag="mrow")
                nc.scalar.activation(out=m_row, in_=pt8[0:4, :], func=EXP,
                                     scale=-0.8)
                m_row4 = sp.tile([1, H, S], BF16, tag="mrow4")
                nc.scalar.dma_start(out=m_row4, in_=m_row[:, None, :])
                m_b = mp.tile([P, H, S], BF16)
                nc.gpsimd.partition_broadcast(m_b[:, 0:2, :], m_row4[:, 0:2, :])
                nc.gpsimd.partition_broadcast(m_b[:, 2:4, :], m_row4[:, 2:4, :])

                # elr + per-source exp scalars
                elr = sp.tile([P, SS, 8], F32, tag="elr")
                nc.vector.tensor_copy(out=elr, in_=pc_all)
                a_bf = sp.tile([P, SS, H], F32, tag="abf")
                c_bf = sp.tile([P, SS, H], F32, tag="cbf")
                nc.scalar.activation(out=a_bf, in_=elr[:, :, 4:8], func=EXP)
                nc.scalar.activation(out=c_bf, in_=elr[:, :, 4:8], func=EXP,
                                     scale=0.2)
                return dict(xt_bf=xt_bf, m_b=m_b, a_bf=a_bf, c_bf=c_bf)

            def emit_proj(ctx):
                """pa/pb matmuls + h-copies (Act) into ha_all."""
                xt_bf = ctx["xt_bf"]
                ha_all = hp.tile([P, SS, H, 257], BF16)
                nc.gpsimd.memset(ha_all[:, :, :, 256:257], 4.0)
                for ss in range(SS):
                    pa = ps_a.tile([P, S], F32, tag="pa")
                    pb = ps_b.tile([P, S], F32, tag="pb")
                    for kc in range(KC):
                        lt = xt_bf[:, kc, ss * P:(ss + 1) * P]
                        st, sp_ = (kc == 0), (kc == KC - 1)
                        nc.tensor.matmul(pa, lhsT=lt, rhs=w_bf[:, kc, 0:512],
                                         start=st, stop=sp_)
                        nc.tensor.matmul(pb, lhsT=lt, rhs=w_bf[:, kc, 512:1024],
                                         start=st, stop=sp_)
                    nc.scalar.copy(out=ha_all[:, ss, 0:2, 0:256],
                                   in_=pa.rearrange("p (h dd) -> p h dd", h=2))
                    nc.scalar.copy(out=ha_all[:, ss, 2:4, 0:256],
                                   in_=pb.rearrange("p (h dd) -> p h dd", h=2))
                ctx["ha_all"] = ha_all

            def emit_expe_chunk(ctx, chunk):
                """expe tiles for (h, ss) pairs in `chunk` (h-major index)."""
                expe = ctx["expe"]
                for idx in chunk:
                    h, ss = idx // SS, idx % SS
                    eng = nc.vector if EXPE_ENG[idx] == "v" else nc.gpsimd
                    eng.tensor_scalar(
                        out=expe[:, h, ss, :],
                        in0=ctx["m_b"][:, h, :],
                        scalar1=ctx["c_bf"][:, ss, h:h + 1],
                        scalar2=ctx["a_bf"][:, ss, h:h + 1],
                        op0=AX.mult, op1=AX.max)

            def emit_agg_dc(ctx, dc):
                """aggregation matmuls + normalize for one dst chunk of the
                PREVIOUS doc."""
                expe, ha_all = ctx["expe"], ctx["ha_all"]
                puA = ps_agg.tile([P, 2, S], F32, tag="puA")
                puB = ps_agg.tile([P, 2, S], F32, tag="puB")
                for hh, pu in ((0, puA), (2, puB)):
                    for h in (hh, hh + 1):
                        for sc in range(SS):
                            nc.tensor.matmul(pu[:, h - hh, 0:257],
                                             lhsT=expe[:, h, sc, dc * P:(dc + 1) * P],
                                             rhs=ha_all[:, sc, h, :],
                                             start=(sc == 0), stop=(sc == SS - 1))
                rzA = sp.tile([P, 2], F32, tag="rzA")
                nc.vector.reciprocal(out=rzA, in_=puA[:, :, 256])
                tmp = sp.tile([P, 2, D], BF16, tag="ntmp")
                nc.scalar.activation(out=tmp[:, 0, :], in_=puA[:, 0, 0:256],
                                     func=CPY, scale=rzA[:, 0:1])
                nc.vector.tensor_scalar(out=tmp[:, 1, :], in0=puA[:, 1, 0:256],
                                        scalar1=rzA[:, 1:2], scalar2=None,
                                        op0=AX.mult)
                rzB = sp.tile([P, 2], F32, tag="rzB")
                nc.vector.reciprocal(out=rzB, in_=puB[:, :, 256])
                tmp2 = sp.tile([P, D], BF16, tag="tmp2")
                nc.scalar.activation(out=tmp2, in_=puB[:, 0, 0:256],
                                     func=CPY, scale=rzB[:, 0:1])
                accd = sp.tile([P, D], F32, tag="accd")
                nc.vector.scalar_tensor_tensor(
                    out=accd, in0=puB[:, 1, 0:256], scalar=rzB[:, 1:2],
                    in1=bias_b, op0=AX.mult, op1=AX.add)
                t01 = sp.tile([P, D], BF16, tag="t01")
                nc.vector.tensor_add(out=t01, in0=tmp[:, 0, :], in1=tmp[:, 1, :])
                t012 = sp.tile([P, D], BF16, tag="t012")
                nc.vector.tensor_add(out=t012, in0=t01, in1=tmp2)
                nc.vector.tensor_add(out=ctx["out_sb"][:, dc, :], in0=t012, in1=accd)

            # expe emission chunks: first 8 tiles (heads 0,1) up front so
            # agg dc0 of THIS doc is unblocked early next iteration; the
            # rest interleave with the previous doc's agg post per dc.
            CHUNKS = [list(range(0, 6)), list(range(6, 10)),
                      list(range(10, 13)), list(range(13, 16)), []]

            for d in range(DPC):
                ctx = emit_front(d)
                expe_t = ep.tile([P, H, SS, S], BF16, tag="expe")
                ctx["expe"] = expe_t
                ctx["out_sb"] = None
                # proj(d) first: its Act h-copies then run during agg(d-1)
                # with a full doc of slack; agg(d-1) on PE overlaps expe(d)
                # generation on DVE/Pool.
                emit_proj(ctx)
                if prev is None:
                    for ch in CHUNKS:
                        emit_expe_chunk(ctx, ch)
                else:
                    out_sb_t = op_pool.tile([P, DC, D], F32, tag="osb")
                    prev["out_sb"] = out_sb_t
                    emit_expe_chunk(ctx, CHUNKS[0])
                    for dc in range(DC):
                        emit_agg_dc(prev, dc)
                        emit_expe_chunk(ctx, CHUNKS[dc + 1])
                    nc.scalar.dma_start(
                        out=out[(d - 1) * S:d * S, :]
                            .rearrange("(dc p) dd -> p dc dd", p=P),
                        in_=prev["out_sb"])
                prev = ctx

            # drain the last doc
            out_sb_last = op_pool.tile([P, DC, D], F32, tag="osb")
            prev["out_sb"] = out_sb_last
            for dc in range(DC):
                emit_agg_dc(prev, dc)
            nc.scalar.dma_start(
                out=out[(DPC - 1) * S:DPC * S, :]
                    .rearrange("(dc p) dd -> p dc dd", p=P),
                in_=prev["out_sb"])


_NC_CACHE = None


def build_nc():
    global _NC_CACHE
    if _NC_CACHE is not None:
        return _NC_CACHE
    nc = bacc.Bacc("TRN2", target_bir_lowering=False, debug=False,
                   num_devices=N_CORES)
    x = nc.dram_tensor("x", [DPC * S, K], F32, kind="ExternalInput")
    w = nc.dram_tensor("w", [P, KC, H * D], BF16, kind="ExternalInput")
    al = nc.dram_tensor("al", [P, KC, 8], BF16, kind="ExternalInput")
    ar = nc.dram_tensor("ar", [P, D], F32, kind="ExternalInput")
    bias_d = nc.dram_tensor("bias", [H * D], F32, kind="ExternalInput")
    out = nc.dram_tensor("out", [DPC * S, K], F32, kind="ExternalOutput")
    with tile.TileContext(nc) as tc:
        gat_tile_kernel(tc, x.ap(), w.ap(), al.ap(), ar.ap(), bias_d.ap(), out.ap())
    nc.compile()
    _NC_CACHE = nc
    return nc


def _host_weight_prep(W, attn_l, attn_r, bias):
    """Device-layout weight constants (host-side weight preprocessing).

    Returns (w_bf [128,2,1024] bf16, wlr_bf [128,2,8] bf16,
    bias_b [128,256] f32).  wlr col layout: cols 0..3 = W @ attn_r per head
    (er), cols 4..7 = W @ attn_l (el) — k rows split [kc, p]."""
    import ml_dtypes
    Wd = W.astype(np.float64)
    w_bf = W.astype(ml_dtypes.bfloat16).reshape(KC, P, H * D) \
        .transpose(1, 0, 2).copy()
    Wr = Wd.reshape(K, H, D)
    wlr = np.empty((K, 8), dtype=np.float64)
    wlr[:, 0:4] = np.einsum("khd,hd->kh", Wr, attn_r.astype(np.float64))
    wlr[:, 4:8] = np.einsum("khd,hd->kh", Wr, attn_l.astype(np.float64))
    wlr_bf = wlr.astype(ml_dtypes.bfloat16).reshape(KC, P, 8) \
        .transpose(1, 0, 2).copy()
    bias_mean = 0.25 * bias.astype(np.float64).reshape(H, D).sum(axis=0)
    bias_b = np.broadcast_to(bias_mean.astype(np.float32), (P, D)).copy()
    return w_bf, wlr_bf, bias_b


def kernel(sent_feature, W, attn_l, attn_r, bias, num_docs=NUM_DOCS, **_unused):
    sent_feature = np.asarray(sent_feature, dtype=np.float32)
    W = np.asarray(W, dtype=np.float32)
    attn_l = np.asarray(attn_l, dtype=np.float32)
    attn_r = np.asarray(attn_r, dtype=np.float32)
    bias = np.asarray(bias, dtype=np.float32)
    w_bf, wlr_bf, bias_b = _host_weight_prep(W, attn_l, attn_r, bias)

    nc = build_nc()
    in_maps = []
    rows = DPC * S
    for c in range(N_CORES):
        in_maps.append({
            "x": sent_feature[c * rows:(c + 1) * rows],
            "w": w_bf, "al": wlr_bf, "ar": bias_b, "bias": bias,
        })
    res = run_bass_kernel_spmd(nc, in_maps, core_ids=list(range(N_CORES)))
    out = np.concatenate([res.results[c]["out"] for c in range(N_CORES)], axis=0)
    return out.astype(np.float32)


# revision 40
# speedup vs baseline: 1.2678x; 1.0244x over previous
"""GAT (DGL GATConv) over complete per-doc graphs — Trainium2 Bass kernel.

Problem: nn_CompletedSentenceGraph (gnn_message_passing).
  64 docs x 512 sentences, HIDDEN=256, HEADS=4, D=256.
  h = (x @ W).reshape(B,S,H,D)
  el/er = einsum(h, attn_l/attn_r)
  e[b,s,t,h] = leaky_relu(el[s]+er[t], 0.2); alpha = softmax over s
  out = einsum(alpha, h) + bias; return mean over heads  -> [N, 256]

Sharding: data-parallel over docs, 8 docs per core on 8 cores.

Math tricks used on-device:
  * exp(lrelu(x)) = max(exp(x), exp(0.2 x)); with x = el_s + er_t both exps
    are rank-1, so per (src,dst) scores need only ONE vector op:
       expe'[s,t] = max(a_s, c_s * m_t),  a=exp(el), c=exp(0.2 el),
       m=exp(-0.8 er)   (score scaled by 1/exp(er_t) per-dst; softmax is
    invariant to per-dst scaling).
  * el/er are computed inside fused matmuls via an augmented weight matrix
    WLR = W @ ALR (ALR block-diagonal from attn_l/attn_r), built on device.
  * Z (softmax denominator) comes free as a ones-column appended to the
    aggregation matmul's rhs; ones are 4.0 so 1/Z also folds the 1/H
    head-mean factor.

Perf structure (v4):
  * PE executes its queue in order, so the doc loop is software-pipelined:
    agg(d-1) is emitted AFTER doc d's pt8/pc/proj matmuls.  While PE runs
    agg(d-1) (~7us), DVE/Pool generate expe(d) and Act does the h-copies
    of proj(d) — so agg(d) can start immediately after.
  * PSUM rings: pa-ring also hosts pt8, pb-ring also hosts pc (saves 2
    banks), leaving pa/pb double-buffered + two 2-bank agg tiles (head
    pairs 0/1 and 2/3) whose drains overlap each other's matmuls.
  * Setup weight transposes go through the DMA XBAR, and setup DMAs use
    the SWDGE (gpsimd) path so the per-doc x pipeline owns SP/HWDGE.
"""

from contextlib import ExitStack

import numpy as np

import concourse.mybir as mybir
import concourse.tile as tile
from concourse import bacc
from concourse.bass_utils import run_bass_kernel_spmd

F32 = mybir.dt.float32
BF16 = mybir.dt.bfloat16
AX = mybir.AluOpType

NUM_DOCS = 64
S = 512          # sentences per doc
K = 256          # hidden
H = 4            # heads
D = 256          # per-head out feats
N_CORES = 8
DPC = NUM_DOCS // N_CORES  # docs per core
P = 128

SS = S // P      # 4 s-subtiles per doc
KC = K // P      # 2 k-chunks
DC = S // P      # 4 dst chunks

EXP = mybir.ActivationFunctionType.Exp
CPY = mybir.ActivationFunctionType.Copy

# expe engine split: 12 DVE / 4 Pool (index = h*SS + ss, h-major order)
EXPE_ENG = ["v"] * 16
for _i in (1, 6, 9, 14):
    EXPE_ENG[_i] = "p"


def gat_tile_kernel(tc, x, w, al, ar, bias_d, out):
    nc = tc.nc

    stack = ExitStack()
    with stack:
        consts = stack.enter_context(tc.tile_pool(name="consts", bufs=1))
        # ---- load host-precomputed weight constants (3 small DMAs) ----
        # w:    [128, 2, 1024] bf16  (k on partitions)
        # al (reused slot): wlr [128, 2, 8] bf16 = W @ ALR, k on partitions
        # ar (reused slot): bias_b [128, 256] f32 broadcast of the head-mean
        wlr_bf = consts.tile([P, KC, 8], BF16)
        nc.scalar.dma_start(out=wlr_bf, in_=al)
        bias_b = consts.tile([P, D], F32)
        nc.scalar.dma_start(out=bias_b, in_=ar)
        w_bf = consts.tile([P, KC, H * D], BF16)
        nc.scalar.dma_start(out=w_bf, in_=w)

        # ---------------- per-doc software-pipelined loop ----------------
        with tc.tile_pool(name="xp", bufs=2) as xp, \
             tc.tile_pool(name="xtp", bufs=2) as xtp, \
             tc.tile_pool(name="hp", bufs=2) as hp, \
             tc.tile_pool(name="ep", bufs=2) as ep, \
             tc.tile_pool(name="mp", bufs=2) as mp, \
             tc.tile_pool(name="sp", bufs=2) as sp, \
             tc.tile_pool(name="op", bufs=2) as op_pool, \
             tc.tile_pool(name="drp", bufs=2, space="DRAM") as drp, \
             tc.tile_pool(name="ps_pp", bufs=2, space="PSUM") as ps_pp, \
             tc.tile_pool(name="ps_agg", bufs=1, space="PSUM") as ps_agg:

            prev = None  # tiles of doc d-1 whose agg is still pending

            def emit_front(d):
                """x load/transpose, logit chain, m_b — everything the
                expe generation of doc d needs, but NOT proj/agg."""
                xd = x[d * S:(d + 1) * S, :]
                x_sb = xp.tile([P, SS, K], F32)
                nc.sync.dma_start(out=x_sb,
                                  in_=xd.rearrange("(ss p) k -> p ss k", p=P))
                x_bf = xp.tile([P, SS, K], BF16, tag="xbf")
                nc.gpsimd.tensor_copy(out=x_bf, in_=x_sb)
                xdr = drp.tile([S, K], BF16)
                nc.sync.dma_start(out=xdr.rearrange("(ss p) k -> p ss k", p=P),
                                  in_=x_bf)
                xt_bf = xtp.tile([P, KC, S], BF16)
                for kc in range(KC):
                    nc.sync.dma_start_transpose(xt_bf[:, kc, :],
                                                xdr[:, kc * P:(kc + 1) * P])

                # pt8 (er/el rows) and pc (el per source) share one pp-ring
                # slot: pt8 in [0:8, 0:512] (bank 0), pc in [:, 512:544]
                # (bank 1) — disjoint regions of a [128, 1024] tile.
                pp0 = ps_pp.tile([P, H * D], F32, tag="pp")
                pt8 = pp0[0:8, 0:S]
                for kc in range(KC):
                    nc.tensor.matmul(pt8, lhsT=wlr_bf[:, kc, :], rhs=xt_bf[:, kc, :],
                                     start=(kc == 0), stop=(kc == KC - 1))
                pc_all = pp0[:, 512:512 + SS * 8].rearrange("p (ss c) -> p ss c", c=8)
                for ss in range(SS):
                    for kc in range(KC):
                        nc.tensor.matmul(pc_all[:, ss, :],
                                         lhsT=xt_bf[:, kc, ss * P:(ss + 1) * P],
                                         rhs=wlr_bf[:, kc, :],
                                         start=(kc == 0), stop=(kc == KC - 1))

                # m_row = exp(-0.8*er); collapse; two 2-head broadcasts
                m_row = sp.tile([4, S], BF16, tag="mrow")
                nc.scalar.activation(out=m_row, in_=pt8[0:4, :], func=EXP,
                                     scale=-0.8)
                m_row4 = sp.tile([1, H, S], BF16, tag="mrow4")
                nc.scalar.dma_start(out=m_row4, in_=m_row[:, None, :])
                m_b = mp.tile([P, H, S], BF16)
                nc.gpsimd.partition_broadcast(m_b[:, 0:2, :], m_row4[:, 0:2, :])
                nc.gpsimd.partition_broadcast(m_b[:, 2:4, :], m_row4[:, 2:4, :])

                # elr + per-source exp scalars
                elr = sp.tile([P, SS, 8], F32, tag="elr")
                nc.vector.tensor_copy(out=elr, in_=pc_all)
                a_bf = sp.tile([P, SS, H], F32, tag="abf")
                c_bf = sp.tile([P, SS, H], F32, tag="cbf")
                nc.scalar.activation(out=a_bf, in_=elr[:, :, 4:8], func=EXP)
                nc.scalar.activation(out=c_bf, in_=elr[:, :, 4:8], func=EXP,
                                     scale=0.2)
                return dict(xt_bf=xt_bf, m_b=m_b, a_bf=a_bf, c_bf=c_bf)

            def emit_proj(ctx):
                """pa/pb matmuls + h-copies (Act) into ha_all."""
                xt_bf = ctx["xt_bf"]
                ha_all = hp.tile([P, SS, H, 257], BF16)
                nc.gpsimd.memset(ha_all[:, :, :, 256:257], 4.0)
                for ss in range(SS):
                    pp = ps_pp.tile([P, H * D], F32, tag="pp")
                    for kc in range(KC):
                        lt = xt_bf[:, kc, ss * P:(ss + 1) * P]
                        st, sp_ = (kc == 0), (kc == KC - 1)
                        nc.tensor.matmul(pp[:, 0:512], lhsT=lt,
                                         rhs=w_bf[:, kc, 0:512], start=st, stop=sp_)
                        nc.tensor.matmul(pp[:, 512:1024], lhsT=lt,
                                         rhs=w_bf[:, kc, 512:1024], start=st, stop=sp_)
                    nc.scalar.copy(out=ha_all[:, ss, :, 0:256],
                                   in_=pp.rearrange("p (h dd) -> p h dd", h=H))
                ctx["ha_all"] = ha_all

            def emit_expe_chunk(ctx, chunk):
                """expe tiles for (h, ss) pairs in `chunk` (h-major index)."""
                expe = ctx["expe"]
                for idx in chunk:
                    h, ss = idx // SS, idx % SS
                    eng = nc.vector if EXPE_ENG[idx] == "v" else nc.gpsimd
                    eng.tensor_scalar(
                        out=expe[:, h, ss, :],
                        in0=ctx["m_b"][:, h, :],
                        scalar1=ctx["c_bf"][:, ss, h:h + 1],
                        scalar2=ctx["a_bf"][:, ss, h:h + 1],
                        op0=AX.mult, op1=AX.max)

            def emit_agg_dc(ctx, dc):
                """aggregation matmuls + normalize for one dst chunk of the
                PREVIOUS doc."""
                expe, ha_all = ctx["expe"], ctx["ha_all"]
                puA = ps_agg.tile([P, 2, S], F32, tag="puA")
                puB = ps_agg.tile([P, 2, S], F32, tag="puB")
                for hh, pu in ((0, puA), (2, puB)):
                    for h in (hh, hh + 1):
                        for sc in range(SS):
                            nc.tensor.matmul(pu[:, h - hh, 0:257],
                                             lhsT=expe[:, h, sc, dc * P:(dc + 1) * P],
                                             rhs=ha_all[:, sc, h, :],
                                             start=(sc == 0), stop=(sc == SS - 1))
                # drain puA: heads 0,1 as Act scaled copies -> bf16
                rzA = sp.tile([P, 2], F32, tag="rzA")
                nc.vector.reciprocal(out=rzA, in_=puA[:, :, 256])
                tmp = sp.tile([P, 2, D], BF16, tag="ntmp")
                for hh in range(2):
                    nc.scalar.activation(out=tmp[:, hh, :], in_=puA[:, hh, 0:256],
                                         func=CPY, scale=rzA[:, hh:hh + 1])
                # drain puB: heads 2,3 as chained DVE STTs (bias folds in)
                rzB = sp.tile([P, 2], F32, tag="rzB")
                nc.vector.reciprocal(out=rzB, in_=puB[:, :, 256])
                acc2 = sp.tile([P, D], F32, tag="acc2")
                nc.vector.scalar_tensor_tensor(
                    out=acc2, in0=puB[:, 0, 0:256], scalar=rzB[:, 0:1],
                    in1=bias_b, op0=AX.mult, op1=AX.add)
                acc3 = sp.tile([P, D], F32, tag="acc3")
                nc.vector.scalar_tensor_tensor(
                    out=acc3, in0=puB[:, 1, 0:256], scalar=rzB[:, 1:2],
                    in1=acc2, op0=AX.mult, op1=AX.add)
                t01 = sp.tile([P, D], BF16, tag="t01")
                nc.vector.tensor_add(out=t01, in0=tmp[:, 0, :], in1=tmp[:, 1, :])
                nc.vector.tensor_add(out=ctx["out_sb"][:, dc, :], in0=t01, in1=acc3)

            # expe emission chunks: first 8 tiles (heads 0,1) up front so
            # agg dc0 of THIS doc is unblocked early next iteration; the
            # rest interleave with the previous doc's agg post per dc.
            CHUNKS = [list(range(0, 6)), list(range(6, 10)),
                      list(range(10, 13)), list(range(13, 16)), []]

            for d in range(DPC):
                ctx = emit_front(d)
                expe_t = ep.tile([P, H, SS, S], BF16, tag="expe")
                ctx["expe"] = expe_t
                ctx["out_sb"] = None
                # agg(d-1) on PE overlaps expe(d) on DVE/Pool; proj(d)
                # after, so its Act h-copies trail into agg(d)'s window
                if prev is None:
                    for ch in CHUNKS:
                        emit_expe_chunk(ctx, ch)
                else:
                    out_sb_t = op_pool.tile([P, DC, D], F32, tag="osb")
                    prev["out_sb"] = out_sb_t
                    emit_expe_chunk(ctx, CHUNKS[0])
                    for dc in range(DC):
                        emit_agg_dc(prev, dc)
                        emit_expe_chunk(ctx, CHUNKS[dc + 1])
                    nc.scalar.dma_start(
                        out=out[(d - 1) * S:d * S, :]
                            .rearrange("(dc p) dd -> p dc dd", p=P),
                        in_=prev["out_sb"])
                emit_proj(ctx)
                prev = ctx

            # drain the last doc
            out_sb_last = op_pool.tile([P, DC, D], F32, tag="osb")
            prev["out_sb"] = out_sb_last
            for dc in range(DC):
                emit_agg_dc(prev, dc)
            nc.scalar.dma_start(
                out=out[(DPC - 1) * S:DPC * S, :]
                    .rearrange("(dc p) dd -> p dc dd", p=P),
                in_=prev["out_sb"])


_NC_CACHE = None


def build_nc():
    global _NC_CACHE
    if _NC_CACHE is not None:
        return _NC_CACHE
    nc = bacc.Bacc("TRN2", target_bir_lowering=False, debug=False,
                   num_devices=N_CORES)
    x = nc.dram_tensor("x", [DPC * S, K], F32, kind="ExternalInput")
    w = nc.dram_tensor("w", [P, KC, H * D], BF16, kind="ExternalInput")
    al = nc.dram_tensor("al", [P, KC, 8], BF16, kind="ExternalInput")
    ar = nc.dram_tensor("ar", [P, D], F32, kind="ExternalInput")
    bias_d = nc.dram_tensor("bias", [H * D], F32, kind="ExternalInput")
    out = nc.dram_tensor("out", [DPC * S, K], F32, kind="ExternalOutput")
    with tile.TileContext(nc) as tc:
        gat_tile_kernel(tc, x.ap(), w.ap(), al.ap(), ar.ap(), bias_d.ap(), out.ap())
    nc.compile()
    _NC_CACHE = nc
    return nc


def _host_weight_prep(W, attn_l, attn_r, bias):
    """Device-layout weight constants (host-side weight preprocessing).

    Returns (w_bf [128,2,1024] bf16, wlr_bf [128,2,8] bf16,
    bias_b [128,256] f32).  wlr col layout: cols 0..3 = W @ attn_r per head
    (er), cols 4..7 = W @ attn_l (el) — k rows split [kc, p]."""
    import ml_dtypes
    Wd = W.astype(np.float64)
    w_bf = W.astype(ml_dtypes.bfloat16).reshape(KC, P, H * D) \
        .transpose(1, 0, 2).copy()
    Wr = Wd.reshape(K, H, D)
    wlr = np.empty((K, 8), dtype=np.float64)
    wlr[:, 0:4] = np.einsum("khd,hd->kh", Wr, attn_r.astype(np.float64))
    wlr[:, 4:8] = np.einsum("khd,hd->kh", Wr, attn_l.astype(np.float64))
    wlr_bf = wlr.astype(ml_dtypes.bfloat16).reshape(KC, P, 8) \
        .transpose(1, 0, 2).copy()
    bias_mean = 0.25 * bias.astype(np.float64).reshape(H, D).sum(axis=0)
    bias_b = np.broadcast_to(bias_mean.astype(np.float32), (P, D)).copy()
    return w_bf, wlr_bf, bias_b


def kernel(sent_feature, W, attn_l, attn_r, bias, num_docs=NUM_DOCS, **_unused):
    sent_feature = np.asarray(sent_feature, dtype=np.float32)
    W = np.asarray(W, dtype=np.float32)
    attn_l = np.asarray(attn_l, dtype=np.float32)
    attn_r = np.asarray(attn_r, dtype=np.float32)
    bias = np.asarray(bias, dtype=np.float32)
    w_bf, wlr_bf, bias_b = _host_weight_prep(W, attn_l, attn_r, bias)

    nc = build_nc()
    in_maps = []
    rows = DPC * S
    for c in range(N_CORES):
        in_maps.append({
            "x": sent_feature[c * rows:(c + 1) * rows],
            "w": w_bf, "al": wlr_bf, "ar": bias_b, "bias": bias,
        })
    res = run_bass_kernel_spmd(nc, in_maps, core_ids=list(range(N_CORES)))
    out = np.concatenate([res.results[c]["out"] for c in range(N_CORES)], axis=0)
    return out.astype(np.float32)
